# revision 1
# baseline (speedup 1.0000x reference)
"""TRN2 Bass kernel for nn_DecoderCell (LFADS-style decoder cell).

Strategy:
  - Pure data parallel: batch 16384 -> 8 cores x 2048 rows.
  - All on-device compute in feature-transposed layout [D, B] so matmuls
    chain on the PE with contraction on partitions; host transposes
    x/h_0 slices in and the output back out (cheap numpy, no HW cost).
  - Weights transposed/interleaved on host into SBUF "walls" (few DMAs);
    per-tile inputs/outputs use k-chunk-interleaved [128, k*B] layouts
    so each tensor moves in one DMA.
  - Matmuls in float32r (fast fp32 path, 1 cycle/row at N=512).
  - Software-pipelined emission: controller+co phases A0..A3 first, then
    generator+factor phases B0..B3, so the PE instruction stream always
    has independent matmuls to run while elementwise epilogues complete.
  - Engine split: matmuls PE, sigmoid/tanh ACT, GRU combine DVE,
    r*h multiplies and clips GPSIMD.
"""
import numpy as np

import concourse.bass as bass
import concourse.bacc as bacc
import concourse.tile as tile
from concourse import mybir
from concourse.bass_utils import run_bass_kernel_spmd
from concourse.bass_interp import get_hw_module

F32 = mybir.dt.float32
F32R = mybir.dt.float32r
AF = mybir.ActivationFunctionType
OP = mybir.AluOpType

GEN, CON, CO, FAC, CI = 512, 256, 128, 128, 128
CLIP = 5.0
EPS = 1e-12
B_FULL = 16384
N_CORES = 8
B_CORE = B_FULL // N_CORES   # 2048
BT = 512                     # batch-tile (free dim) per pipeline step
NT = B_CORE // BT            # 4

# wallA column offsets (controller weights)
# [wihc z/r (2k x 512), wihc n (2k x 256), whhc (2k x 768), cow (2k x 256)]
OFF_WIHC_N = 1024
OFF_WHHC = 1536
OFF_COW = 3072
WALLA_COLS = 3584
# wallB column offsets (generator weights)
OFF_GIH = 0           # 1 x 1536
OFF_GHH = 1536        # 4 k-chunks x 1536
WALLB_COLS = 1536 + 4 * 1536

ts = bass.ts


def build_program(repeats=1):
    nc = bacc.Bacc("TRN2", target_bir_lowering=False, debug=False)

    # ---- DRAM I/O (per-core shapes) ----
    xT = nc.dram_tensor("xT", [CI, B_CORE], F32R, kind="ExternalInput")
    genT = nc.dram_tensor("genT", [128, 4 * B_CORE], F32R, kind="ExternalInput")
    conT = nc.dram_tensor("conT", [128, 2 * B_CORE], F32R, kind="ExternalInput")
    facT = nc.dram_tensor("facT", [FAC, B_CORE], F32R, kind="ExternalInput")
    wallA = nc.dram_tensor("wallA", [128, WALLA_COLS], F32R, kind="ExternalInput")
    wallB = nc.dram_tensor("wallB", [128, WALLB_COLS], F32R, kind="ExternalInput")
    facw = nc.dram_tensor("facw", [128, 4 * FAC], F32, kind="ExternalInput")
    biasd = nc.dram_tensor("biasd", [128, 26], F32, kind="ExternalInput")

    genO = nc.dram_tensor("genO", [128, 4 * B_CORE], F32R, kind="ExternalOutput")
    conO = nc.dram_tensor("conO", [128, 2 * B_CORE], F32R, kind="ExternalOutput")
    coO = nc.dram_tensor("coO", [128, 2 * B_CORE], F32R, kind="ExternalOutput")
    facO = nc.dram_tensor("facO", [FAC, B_CORE], F32, kind="ExternalOutput")

    with tile.TileContext(nc) as tc:
        with (
            tc.tile_pool(name="wpool", bufs=1) as wpool,
            tc.tile_pool(name="inp", bufs=1) as inp,
            tc.tile_pool(name="zrA", bufs=2) as zrA,
            tc.tile_pool(name="zrB", bufs=1) as zrB,
            tc.tile_pool(name="midA", bufs=2) as midA,
            tc.tile_pool(name="midB", bufs=1) as midB,
            tc.tile_pool(name="outA", bufs=2) as outA,
            tc.tile_pool(name="outCO", bufs=4) as outCO,
            tc.tile_pool(name="outB", bufs=2) as outB,
            tc.tile_pool(name="psum", bufs=8, space="PSUM") as psum,
        ):
            # ---- controller weights ----
            wa = wpool.tile([128, WALLA_COLS], F32R, tag="wallA")
            # z/r-gate columns of con weights first so the first matmuls
            # can start after ~1MB of DMA
            nc.sync.dma_start(wa[:, 0:1024], wallA[:, 0:1024])
            bias_sb = wpool.tile([128, 26], F32, tag="bias")
            nc.sync.dma_start(bias_sb[:], biasd[:])

            def bias_ap(col):
                return bias_sb[:, col:col + 1]

            def wihc(k, gate, c):
                if gate < 2:
                    o = k * 512 + gate * CON + c * 128
                else:
                    o = OFF_WIHC_N + k * 256 + c * 128
                return wa[:, o:o + 128]

            def whhc(k, gate, c):
                o = OFF_WHHC + k * 768 + gate * CON + c * 128
                return wa[:, o:o + 128]

            def coww(k, c):
                o = OFF_COW + k * 256 + c * 128
                return wa[:, o:o + 128]

            st = [dict() for _ in range(NT)]   # per-tile state
            wb = wpool.tile([128, WALLB_COLS], F32R, tag="wallB")

            def gih(gate, c):
                o = OFF_GIH + gate * GEN + c * 128
                return wb[:, o:o + 128]

            def ghh(k, gate, c):
                o = OFF_GHH + k * 1536 + gate * GEN + c * 128
                return wb[:, o:o + 128]

            def emit_inputs(bt):
                bs = ts(bt, BT)
                s = st[bt]
                s["xt"] = inp.tile([128, BT], F32R, tag="xt", name=f"xt{bt}", bufs=4)
                nc.sync.dma_start(s["xt"][:], xT[:, bs])
                s["ft"] = inp.tile([128, BT], F32R, tag="ft", name=f"ft{bt}", bufs=4)
                nc.sync.dma_start(s["ft"][:], facT[:, bs])
                s["ct"] = inp.tile([128, 2 * BT], F32R, tag="ct", name=f"ct{bt}", bufs=4)
                nc.sync.dma_start(
                    s["ct"][:].rearrange("p (k b) -> p k b", k=2),
                    conT.rearrange("p (k b) -> p k b", k=2)[:, :, bs])

            def G1(bt):
                # controller z and r gates: matmuls + sigmoids + r*h.
                # zc = sigmoid(-pre) = 1-z via a second ACT read of the psum;
                # zh = z*h is hoisted here so only zc*n + add + clip remain
                # after the tanh.
                s = st[bt]
                xt, ft, ct = s["xt"], s["ft"], s["ct"]
                z, r, zps = [], [], []
                for gi_, (lst, dt_, bcol) in enumerate(
                    ((z, F32, 0), (r, F32R, 2))
                ):
                    for c in range(2):
                        ps = psum.tile([128, BT], F32, tag="ps", name=f"psc{bt}{gi_}{c}")
                        nc.tensor.matmul(ps[:], wihc(0, gi_, c), xt[:],
                                         start=True, stop=False)
                        nc.tensor.matmul(ps[:], wihc(1, gi_, c), ft[:],
                                         start=False, stop=False)
                        nc.tensor.matmul(ps[:], whhc(0, gi_, c), ct[:, ts(0, BT)],
                                         start=False, stop=False)
                        nc.tensor.matmul(ps[:], whhc(1, gi_, c), ct[:, ts(1, BT)],
                                         start=False, stop=True)
                        g = zrA.tile([128, BT], dt_, tag=f"czr{gi_}{c}",
                                     name=f"czr{bt}{gi_}{c}")
                        nc.scalar.activation(g[:], ps[:], AF.Sigmoid,
                                             bias=bias_ap(bcol + c))
                        lst.append(g)
                        if gi_ == 0:
                            zps.append(ps)
                for c in range(2):
                    nc.gpsimd.tensor_tensor(
                        r[c][:], r[c][:].bitcast(F32), ct[:, ts(c, BT)].bitcast(F32),
                        OP.mult)
                zc = []
                for c in range(2):
                    t = zrA.tile([128, BT], F32, tag=f"czc{c}", name=f"czc{bt}{c}")
                    nc.scalar.activation(t[:], zps[c][:], AF.Sigmoid,
                                         bias=bias_ap(20 + c), scale=-1.0)
                    zc.append(t)
                    # z becomes z*h in place
                    nc.vector.tensor_tensor(z[c][:], z[c][:],
                                            ct[:, ts(c, BT)].bitcast(F32), OP.mult)
                s["z"], s["r"], s["zc"] = z, r, zc

            def G2(bt):
                # controller n gate + combine + clip + conO store
                bs = ts(bt, BT)
                s = st[bt]
                xt, ft, ct, z, r = s["xt"], s["ft"], s["ct"], s["z"], s["r"]
                conOut = outA.tile([128, 2 * BT], F32R, tag="conOut",
                                   name=f"conOut{bt}")
                s["conOut"] = conOut
                n = []
                for c in range(2):
                    ps = psum.tile([128, BT], F32, tag="ps", name=f"psn{bt}{c}")
                    nc.tensor.matmul(ps[:], wihc(0, 2, c), xt[:],
                                     start=True, stop=False)
                    nc.tensor.matmul(ps[:], wihc(1, 2, c), ft[:],
                                     start=False, stop=False)
                    nc.tensor.matmul(ps[:], whhc(0, 2, c), r[0][:],
                                     start=False, stop=False)
                    nc.tensor.matmul(ps[:], whhc(1, 2, c), r[1][:],
                                     start=False, stop=True)
                    t = midA.tile([128, BT], F32, tag=f"cn{c}", name=f"cn{bt}{c}")
                    nc.scalar.activation(t[:], ps[:], AF.Tanh, bias=bias_ap(4 + c))
                    n.append(t)
                zc = s["zc"]
                for c in range(2):
                    o = conOut[:, ts(c, BT)]
                    nc.vector.tensor_tensor(zc[c][:], zc[c][:], n[c][:], OP.mult)
                    nc.vector.tensor_tensor(o, z[c][:], zc[c][:], OP.add)
                    nc.gpsimd.tensor_scalar(o, o, CLIP, -CLIP, OP.min, OP.max)
                nc.sync.dma_start(
                    conO.rearrange("p (k b) -> p k b", k=2)[:, :, bs],
                    conOut[:].rearrange("p (k b) -> p k b", k=2))

            def G3(bt):
                # co linear + coO store
                s = st[bt]
                conOut = s["conOut"]
                gi_t = outCO.tile([128, BT], F32R, tag="giT", name=f"giT{bt}")
                colog = outA.tile([128, BT], F32R, tag="colog", name=f"colog{bt}")
                s["gi"] = gi_t
                for c, dst in ((0, gi_t), (1, colog)):
                    ps = psum.tile([128, BT], F32, tag="ps", name=f"psco{bt}{c}")
                    nc.tensor.matmul(ps[:], coww(0, c), conOut[:, ts(0, BT)],
                                     start=True, stop=False)
                    nc.tensor.matmul(ps[:], coww(1, c), conOut[:, ts(1, BT)],
                                     start=False, stop=True)
                    nc.vector.tensor_scalar_add(dst[:], ps[:], bias_ap(6 + c))
                    nc.sync.dma_start(
                        coO[:, c * B_CORE + bt * BT:c * B_CORE + (bt + 1) * BT],
                        dst[:])

            def emit_gt(bt):
                bs = ts(bt, BT)
                s = st[bt]
                gt = inp.tile([128, 4 * BT], F32R, tag="gt", name=f"gt{bt}", bufs=2)
                nc.sync.dma_start(
                    gt[:].rearrange("p (k b) -> p k b", k=4),
                    genT.rearrange("p (k b) -> p k b", k=4)[:, :, bs])
                s["gt"] = gt

            def G45(bt):
                # generator z and r gates + sigmoids + r*h
                s = st[bt]
                gt = s["gt"]
                gi_r = s["gi"][:]
                zg, rg, zps = [], [], []
                for gi_, (lst, dt_, bcol) in enumerate(
                    ((zg, F32, 8), (rg, F32R, 12))
                ):
                    for c in range(4):
                        ps = psum.tile([128, BT], F32, tag="ps",
                                       name=f"psg{bt}{gi_}{c}")
                        for k in range(4):
                            nc.tensor.matmul(ps[:], ghh(k, gi_, c), gt[:, ts(k, BT)],
                                             start=(k == 0), stop=False)
                        nc.tensor.matmul(ps[:], gih(gi_, c), gi_r,
                                         start=False, stop=True)
                        g = zrB.tile([128, BT], dt_, tag=f"gzr{gi_}{c}",
                                     name=f"gzr{bt}{gi_}{c}")
                        nc.scalar.activation(g[:], ps[:], AF.Sigmoid,
                                             bias=bias_ap(bcol + c))
                        lst.append(g)
                        if gi_ == 0:
                            zps.append(ps)
                for k in range(4):
                    nc.gpsimd.tensor_tensor(
                        rg[k][:], rg[k][:].bitcast(F32), gt[:, ts(k, BT)].bitcast(F32),
                        OP.mult)
                zc = []
                for c in range(4):
                    t = zrB.tile([128, BT], F32, tag=f"gzc{c}", name=f"gzc{bt}{c}")
                    nc.scalar.activation(t[:], zps[c][:], AF.Sigmoid,
                                         bias=bias_ap(22 + c), scale=-1.0)
                    zc.append(t)
                    nc.vector.tensor_tensor(zg[c][:], zg[c][:],
                                            gt[:, ts(c, BT)].bitcast(F32), OP.mult)
                s["zg"], s["rg"], s["zcg"] = zg, rg, zc

            def G6(bt):
                # generator n gate + combine + clip + genO store
                bs = ts(bt, BT)
                s = st[bt]
                gt, zg, rg = s["gt"], s["zg"], s["rg"]
                gi_r = s["gi"][:]
                genOut = outB.tile([128, 4 * BT], F32R, tag="genOut",
                                   name=f"genOut{bt}")
                s["genOut"] = genOut
                ng = []
                for c in range(4):
                    ps = psum.tile([128, BT], F32, tag="ps", name=f"psgn{bt}{c}")
                    nc.tensor.matmul(ps[:], gih(2, c), gi_r,
                                     start=True, stop=False)
                    for k in range(4):
                        nc.tensor.matmul(ps[:], ghh(k, 2, c), rg[k][:],
                                         start=False, stop=(k == 3))
                    t = midB.tile([128, BT], F32, tag=f"gn{c}", name=f"gn{bt}{c}")
                    nc.scalar.activation(t[:], ps[:], AF.Tanh, bias=bias_ap(16 + c))
                    ng.append(t)
                zc = s["zcg"]
                for c in range(4):
                    o = genOut[:, ts(c, BT)]
                    nc.vector.tensor_tensor(zc[c][:], zc[c][:], ng[c][:], OP.mult)
                    nc.vector.tensor_tensor(o, zg[c][:], zc[c][:], OP.add)
                    nc.gpsimd.tensor_scalar(o, o, CLIP, -CLIP, OP.min, OP.max)
                    if bt == NT - 1:
                        nc.sync.dma_start(
                            genO[:, c * B_CORE + bt * BT:c * B_CORE + (bt + 1) * BT],
                            o)
                if bt != NT - 1:
                    nc.sync.dma_start(
                        genO.rearrange("p (k b) -> p k b", k=4)[:, :, bs],
                        genOut[:].rearrange("p (k b) -> p k b", k=4))

            def G7(bt, facn):
                # factor projection + facO store
                bs = ts(bt, BT)
                s = st[bt]
                genOut = s["genOut"]
                ps = psum.tile([128, BT], F32, tag="ps", name=f"psf{bt}")
                for k in range(4):
                    nc.tensor.matmul(ps[:], facn[:, ts(k, FAC)], genOut[:, ts(k, BT)],
                                     start=(k == 0), stop=(k == 3))
                fo = midB.tile([128, BT], F32, tag="fo", name=f"fo{bt}")
                nc.vector.tensor_copy(fo[:], ps[:])
                nc.sync.dma_start(facO[:, bs], fo[:])

            # ---- emission schedule (modulo software pipeline) ----
            # All loads issued upfront in priority order: the SP queue is
            # in-order, so a compute-gated store emitted early would block
            # every later load (head-of-line).
            for _rep in range(repeats):
              if True:
                emit_inputs(0)
              if _rep == 0:
                  nc.sync.dma_start(wa[:, 1024:WALLA_COLS],
                                    wallA[:, 1024:WALLA_COLS])
                  fraw = wpool.tile([128, 4 * FAC], F32, tag="fraw")
                  nc.sync.dma_start(fraw[:], facw[:])
              emit_inputs(1)
              emit_inputs(2)
              emit_inputs(3)
              if _rep == 0:
                  nc.sync.dma_start(
                      wb[:].rearrange("p (blk col) -> p blk col", col=1536)[:, :, 0:1024],
                      wallB.rearrange("p (blk col) -> p blk col", col=1536)[:, :, 0:1024])
              emit_gt(0)
              emit_gt(1)
              if _rep == 0:
                  nc.sync.dma_start(
                      wb[:].rearrange("p (blk col) -> p blk col", col=1536)[:, :, 1024:1536],
                      wallB.rearrange("p (blk col) -> p blk col", col=1536)[:, :, 1024:1536])
              G1(0)
              G1(1)
              G2(0)
              G1(2)
              G2(1)
              G3(0)
              G1(3)
              G2(2)
              G3(1)
              if _rep == 0:
                  # fac_w normalization (tiny)
                  fsq = wpool.tile([128, FAC], F32, tag="fsq")
                  fss = wpool.tile([128, 4], F32, tag="fss")
                  for k in range(4):
                      nc.vector.scalar_tensor_tensor(
                          fsq[:], fraw[:, ts(k, FAC)], 1.0, fraw[:, ts(k, FAC)],
                          OP.mult, OP.mult, accum_out=fss[:, k:k + 1])
                  fnr = wpool.tile([128, 4], F32, tag="fnr")
                  nc.scalar.activation(fnr[:], fss[:], AF.Sqrt)
                  nc.vector.tensor_scalar_max(fnr[:], fnr[:], EPS)
                  frc = wpool.tile([128, 4], F32, tag="frc")
                  nc.vector.reciprocal(frc[:], fnr[:])
                  facn = wpool.tile([128, 4 * FAC], F32R, tag="facn")
                  for k in range(4):
                      nc.vector.tensor_scalar_mul(
                          facn[:, ts(k, FAC)], fraw[:, ts(k, FAC)],
                          frc[:, k:k + 1])

              G2(3)
              G45(0)
              G3(2)
              G6(0)
              G3(3)
              emit_gt(2)
              G45(1)
              G7(0, facn)
              G6(1)
              emit_gt(3)
              G45(2)
              G7(1, facn)
              G6(2)
              G45(3)
              G7(2, facn)
              G6(3)
              G7(3, facn)

    nc.compile()
    nc.finalize()
    return nc


_NC = None


def _get_nc():
    global _NC
    if _NC is None:
        nc = build_program()
        nc.m = get_hw_module(nc.m)
        _NC = nc
    return _NC


def _interleave_kchunks(wT, k):
    """[k*128, M] -> [128, k*M] with chunk k side by side."""
    m = wT.shape[1]
    return np.ascontiguousarray(
        wT.reshape(k, 128, m).transpose(1, 0, 2).reshape(128, k * m))


def _prep_shared(con_w_ih, con_b_ih, con_w_hh, con_b_hh, co_w, co_b,
                 gen_w_ih, gen_b_ih, gen_w_hh, gen_b_hh, fac_w):
    f32 = np.float32
    wihcT = np.ascontiguousarray(con_w_ih.T, dtype=f32)
    wihc_zr = np.concatenate([wihcT[0:128, 0:512], wihcT[128:256, 0:512]], axis=1)
    wihc_n = np.concatenate([wihcT[0:128, 512:768], wihcT[128:256, 512:768]], axis=1)
    whhc = _interleave_kchunks(np.ascontiguousarray(con_w_hh.T, dtype=f32), 2)
    cow = _interleave_kchunks(np.ascontiguousarray(co_w.T, dtype=f32), 2)
    gihw = np.ascontiguousarray(gen_w_ih.T, dtype=f32)
    ghhw = _interleave_kchunks(np.ascontiguousarray(gen_w_hh.T, dtype=f32), 4)
    shared = {
        "wallA": np.ascontiguousarray(
            np.concatenate([wihc_zr, wihc_n, whhc, cow], axis=1)),
        "wallB": np.ascontiguousarray(np.concatenate([gihw, ghhw], axis=1)),
        "facw": _interleave_kchunks(np.ascontiguousarray(fac_w.T, dtype=f32), 4),
    }
    bias = np.zeros((128, 26), dtype=f32)
    bz = con_b_ih[0:256] + con_b_hh[0:256]
    br = con_b_ih[256:512] + con_b_hh[256:512]
    bn = con_b_ih[512:768] + con_b_hh[512:768]
    for c in range(2):
        bias[:, 0 + c] = bz[c * 128:(c + 1) * 128]
        bias[:, 2 + c] = br[c * 128:(c + 1) * 128]
        bias[:, 4 + c] = bn[c * 128:(c + 1) * 128]
        bias[:, 6 + c] = co_b[c * 128:(c + 1) * 128]
    bzg = gen_b_ih[0:512] + gen_b_hh[0:512]
    brg = gen_b_ih[512:1024] + gen_b_hh[512:1024]
    bng = gen_b_ih[1024:1536] + gen_b_hh[1024:1536]
    for c in range(4):
        bias[:, 8 + c] = bzg[c * 128:(c + 1) * 128]
        bias[:, 12 + c] = brg[c * 128:(c + 1) * 128]
        bias[:, 16 + c] = bng[c * 128:(c + 1) * 128]
    bias[:, 20:22] = -bias[:, 0:2]    # -bz_con
    bias[:, 22:26] = -bias[:, 8:12]   # -bz_gen
    shared["biasd"] = bias
    return shared


def _deinterleave(arr, k):
    """[128, k*B] -> [k*128, B]"""
    b = arr.shape[1] // k
    return arr.reshape(128, k, b).transpose(1, 0, 2).reshape(k * 128, b)


def kernel(x, h_0, con_w_ih, con_b_ih, con_w_hh, con_b_hh, co_w, co_b,
           gen_w_ih, gen_b_ih, gen_w_hh, gen_b_hh, fac_w):
    nc = _get_nc()
    x = np.asarray(x, dtype=np.float32)
    h_0 = np.asarray(h_0, dtype=np.float32)
    shared = _prep_shared(
        np.asarray(con_w_ih), np.asarray(con_b_ih), np.asarray(con_w_hh),
        np.asarray(con_b_hh), np.asarray(co_w), np.asarray(co_b),
        np.asarray(gen_w_ih), np.asarray(gen_b_ih), np.asarray(gen_w_hh),
        np.asarray(gen_b_hh), np.asarray(fac_w))

    in_maps = []
    for c in range(N_CORES):
        s, e = c * B_CORE, (c + 1) * B_CORE
        m = dict(shared)
        m["xT"] = np.ascontiguousarray(x[s:e, :CI].T)
        m["genT"] = _interleave_kchunks(
            np.ascontiguousarray(h_0[s:e, 0:GEN].T), 4)
        m["conT"] = _interleave_kchunks(
            np.ascontiguousarray(h_0[s:e, GEN:GEN + CON].T), 2)
        m["facT"] = np.ascontiguousarray(h_0[s:e, GEN + CON + 3 * CO:].T)
        in_maps.append(m)

    res = run_bass_kernel_spmd(nc, in_maps, core_ids=list(range(N_CORES)))

    out = np.empty((B_FULL, 1280), dtype=np.float32)
    for c in range(N_CORES):
        s, e = c * B_CORE, (c + 1) * B_CORE
        r = res.results[c]
        out[s:e, 0:GEN] = _deinterleave(r["genO"], 4).T
        out[s:e, GEN:GEN + CON] = _deinterleave(r["conO"], 2).T
        co2 = _deinterleave(r["coO"], 2)
        out[s:e, 768:1024] = co2.T
        out[s:e, 1024:1152] = co2[0:CO].T
        out[s:e, 1152:1280] = r["facO"].T
    return out

